# revision 19
# baseline (speedup 1.0000x reference)
"""Multi-head attention (B=4, N=2048, D=768, H=8) on 8 TRN2 NeuronCores.

Sharding: core c = (bg, hg) = divmod(c, 4) handles batches {2bg, 2bg+1} and
heads {2hg, 2hg+1}.  The reference reshapes [B,H,N,dh] -> [B,N,D] without a
head transpose, so output row r of batch b depends only on head r//256 —
each core's projection output is a contiguous [2, 512, 768] block of the
final output and no cross-core communication is needed.

Per-core pipeline (all in fp32 memory; matmuls optionally run as fp32r):
  1. x  -> xT via PE transposes (contraction over D needs D on partitions)
  2. QT/KT/VT = W.T @ xT   (96-row outputs, d on partitions)
  3. scoresT[k,q] = K @ Q.T   (k on partitions, q free)
  4. attnT = exp(scale * scoresT)  on ScalarE (scores are O(6), no max needed)
  5. attnoutT[d,q] (+ row of softmax sums) = [V | ones].T-style accumulation:
     lhsT = V-natural chunks with a ones column appended -> row 96 of the
     psum accumulator is the softmax denominator for free.
  6. normalize: reciprocal (DVE) + partition_broadcast (GPSIMD) + multiply
  7. projection: final[r,:] = sum_j attnoutT[:, (r%256)*8+j] @ W_o[j*96:,:]
     with bias added via a K=1 ones matmul into the same psum group.
"""

import os

import numpy as np

import concourse.bacc as bacc
import concourse.bass as bass
import concourse.mybir as mybir
import concourse.tile as tile
from concourse.bass_utils import run_bass_kernel_spmd
from concourse.masks import make_identity

F32 = mybir.dt.float32
AF = mybir.ActivationFunctionType

B = 4
N = 2048
D = 768
H = 8
DH = 96
B_LOC = 2  # batches per core
H_LOC = 2  # heads per core
NT = 512  # n-tile (columns of xT processed per qkv step)
SCALE = float(DH) ** -0.5

# matmul input dtype: float32r streams 1 row/cycle (vs 4 for float32) when
# the moving free dim is >= 256.  Memory layout is identical to float32.
MM_DT_NAME = os.environ.get("BASS_MM_DT", "float32r")


def _env(name, default):
    return int(os.environ.get(name, default))


def _build(mm_dt: mybir.dt) -> bass.Bass:
    nc = bacc.Bacc(
        "TRN2",
        target_bir_lowering=False,
        debug=False,
        enable_asserts=False,
        num_devices=8,
    )

    CDT = mm_dt  # dtype of SBUF tiles that feed matmuls (np-identical to f32)
    x = nc.dram_tensor("x", [B_LOC, N, D], F32, kind="ExternalInput").ap()
    wq = nc.dram_tensor("wq", [D, H_LOC * DH], CDT, kind="ExternalInput").ap()
    wk = nc.dram_tensor("wk", [D, H_LOC * DH], CDT, kind="ExternalInput").ap()
    wv = nc.dram_tensor("wv", [D, H_LOC * DH], CDT, kind="ExternalInput").ap()
    wo = nc.dram_tensor("wo", [D, D], CDT, kind="ExternalInput").ap()
    bo = nc.dram_tensor("bo", [1, D], CDT, kind="ExternalInput").ap()
    y = nc.dram_tensor("y", [B_LOC, H_LOC * 256, D], F32, kind="ExternalOutput").ap()

    with tile.TileContext(nc) as tc:
        with (
            tc.tile_pool(name="const", bufs=1) as const,
            tc.tile_pool(name="xload", bufs=2) as xload,
            tc.tile_pool(name="xTp", bufs=2) as xTp,
            tc.tile_pool(name="qkp", bufs=_env("QK_BUFS", 2)) as qkp,
            tc.tile_pool(name="vp", bufs=_env("VT_BUFS", 2)) as vp,
            tc.tile_pool(name="vstp", bufs=_env("VST_BUFS", 1)) as vstp,
            tc.tile_pool(name="atp", bufs=_env("AT_BUFS", 2)) as atp,
            tc.tile_pool(name="aop", bufs=_env("AO_BUFS", 1)) as aop,
            tc.tile_pool(name="rp", bufs=_env("RP_BUFS", 1)) as rp,
            tc.tile_pool(name="rbp", bufs=_env("RB_BUFS", 1)) as rbp,
            tc.tile_pool(name="obp", bufs=_env("OB_BUFS", 2)) as obp,
            tc.tile_pool(name="psA", bufs=_env("PSA_BUFS", 2), space="PSUM") as psA,
            tc.tile_pool(name="psQ", bufs=_env("PSQ_BUFS", 2), space="PSUM") as psQ,
            tc.tile_pool(name="psS", bufs=_env("PSS_BUFS", 2), space="PSUM") as psS,
            tc.tile_pool(name="psAcc", bufs=_env("PSACC_BUFS", 2), space="PSUM") as psAcc,
        ):
            ident = const.tile([128, 128], F32, name="ident")
            make_identity(nc, ident)
            ones_f32 = const.tile([1, 128], F32, name="ones_f32")
            nc.gpsimd.memset(ones_f32, 1.0)
            ones_l = const.tile([1, 128], CDT, name="ones_l")
            nc.vector.tensor_copy(ones_l, ones_f32)
            ones16 = const.tile([128, 16], F32, name="ones16")
            nc.gpsimd.memset(ones16, 1.0)
            bo_t = const.tile([1, D], CDT, name="bo_t")
            nc.gpsimd.dma_start(bo_t, bo)

            w_tiles = {}
            for wnm, wap in (("q", wq), ("k", wk), ("v", wv)):
                for kk in range(6):
                    t = const.tile([128, H_LOC * DH], CDT, name=f"w{wnm}{kk}")
                    nc.gpsimd.dma_start(t, wap[kk * 128 : (kk + 1) * 128, :])
                    w_tiles[wnm, kk] = t
            wo_t = []
            for j in range(8):
                t = const.tile([96, D], CDT, name=f"wo{j}")
                nc.gpsimd.dma_start(t, wo[j * 96 : (j + 1) * 96, :])
                wo_t.append(t)

            for b in range(B_LOC):
                # ---- stage A: xT + QKV for both heads --------------------
                qt = {}
                kt = {}
                vt = {}
                ao = {}
                for h in range(H_LOC):
                    qt[h] = qkp.tile([96, N], CDT, name=f"qt{h}", tag=f"qt{h}")
                    kt[h] = qkp.tile([96, N], CDT, name=f"kt{h}", tag=f"kt{h}")
                    vt[h] = vp.tile([128, 16 * 97], CDT, name=f"vt{h}", tag=f"vt{h}")
                    # ones column for the softmax-denominator row
                    nc.vector.tensor_copy(vt[h][:, 96 : 16 * 97 : 97], ones16)
                    ao[h] = aop.tile([96, N], CDT, name=f"ao{h}", tag=f"ao{h}")

                for nt in range(N // NT):
                    xl = xload.tile([128, 4 * D], F32, name="xl", tag="xl")
                    for s in range(4):
                        nc.sync.dma_start(
                            xl[:, s * D : (s + 1) * D],
                            x[b, nt * NT + s * 128 : nt * NT + (s + 1) * 128, :],
                        )
                    xT = xTp.tile([128, 6 * NT], CDT, name="xT", tag="xT")
                    for kk in range(6):
                        pst = psA.tile([128, NT], F32, name="pst", tag="big")
                        for s in range(4):
                            nc.tensor.transpose(
                                pst[:, s * 128 : (s + 1) * 128],
                                xl[:, s * D + kk * 128 : s * D + (kk + 1) * 128],
                                ident,
                            )
                        nc.vector.tensor_copy(xT[:, kk * NT : (kk + 1) * NT], pst)

                    for h in range(H_LOC):
                        for wnm in ("q", "k", "v"):
                            ps = psQ.tile([96, NT], F32, name="psq", tag="qkv")
                            for kk in range(6):
                                nc.tensor.matmul(
                                    ps,
                                    w_tiles[wnm, kk][:, h * DH : (h + 1) * DH],
                                    xT[:, kk * NT : (kk + 1) * NT],
                                    start=(kk == 0),
                                    stop=(kk == 5),
                                )
                            if wnm == "q":
                                nc.vector.tensor_copy(
                                    qt[h][:, nt * NT : (nt + 1) * NT], ps
                                )
                            elif wnm == "k":
                                nc.vector.tensor_copy(
                                    kt[h][:, nt * NT : (nt + 1) * NT], ps
                                )
                            else:
                                vstage = vstp.tile([96, NT], F32, name="vstage", tag="vst")
                                nc.vector.tensor_copy(vstage, ps)
                                pvt = psA.tile([128, 4 * 97], F32, name="pvt", tag="big")
                                for s in range(4):
                                    nc.tensor.transpose(
                                        pvt[:, s * 97 : s * 97 + 96],
                                        vstage[:, s * 128 : (s + 1) * 128],
                                        ident[:96, :96],
                                    )
                                # copy only the 96 data cols of each 97-block
                                # (col 96 of each block is the ones column)
                                nc.vector.tensor_copy(
                                    vt[h][:, nt * 4 * 97 : (nt * 4 + 4) * 97]
                                    .rearrange("p (s c) -> p s c", s=4, c=97)[
                                        :, :, 0:96
                                    ],
                                    pvt.rearrange("p (s c) -> p s c", s=4, c=97)[
                                        :, :, 0:96
                                    ],
                                )

                # ---- stages B+C interleaved per head ---------------------
                # (proj of head h gives PE dense work while the next head's
                #  ACT-paced attention stretch runs)
                for h in range(H_LOC):
                    for qi in range(N // NT):
                        pacc = psAcc.tile([97, NT], F32, name="pacc", tag="acc")
                        for ki in range(N // 128):
                            pss = psS.tile([128, NT], F32, name="pss", tag="s")
                            nc.tensor.matmul(
                                pss,
                                kt[h][:, ki * 128 : (ki + 1) * 128],
                                qt[h][:, qi * NT : (qi + 1) * NT],
                                start=True,
                                stop=True,
                            )
                            at = atp.tile([128, NT], CDT, name="at", tag="at")
                            nc.scalar.activation(at, pss, AF.Exp, scale=SCALE)
                            nc.tensor.matmul(
                                pacc,
                                vt[h][:, ki * 97 : (ki + 1) * 97],
                                at,
                                start=(ki == 0),
                                stop=(ki == 15),
                            )
                        rr = rp.tile([1, NT], F32, name="rr", tag="rr")
                        nc.vector.reciprocal(rr, pacc[96:97, :])
                        rb = rbp.tile([96, NT], F32, name="rb", tag="rb")
                        nc.gpsimd.partition_broadcast(rb, rr)
                        nc.vector.tensor_mul(
                            ao[h][:, qi * NT : (qi + 1) * NT], pacc[0:96, :], rb
                        )

                    # ---- output projection for head h --------------------
                    aor = ao[h].rearrange("p (m r j) -> p m r j", m=2, r=128, j=8)
                    for m in range(2):
                        for nn in range(2):
                            pp = psA.tile([128, 384], F32, name="pp", tag="big")
                            for j in range(8):
                                nc.tensor.matmul(
                                    pp,
                                    aor[:, m, :, j],
                                    wo_t[j][:, nn * 384 : (nn + 1) * 384],
                                    start=(j == 0),
                                    stop=False,
                                )
                            nc.tensor.matmul(
                                pp,
                                ones_l,
                                bo_t[:, nn * 384 : (nn + 1) * 384],
                                start=False,
                                stop=True,
                            )
                            ob = obp.tile([128, 384], F32, name="ob", tag="ob")
                            nc.vector.tensor_copy(ob, pp)
                            nc.sync.dma_start(
                                y[
                                    b,
                                    h * 256 + m * 128 : h * 256 + (m + 1) * 128,
                                    nn * 384 : (nn + 1) * 384,
                                ],
                                ob,
                            )
    nc.compile()
    return nc


_NC_CACHE: dict[str, bass.Bass] = {}


def _get_nc() -> bass.Bass:
    if MM_DT_NAME not in _NC_CACHE:
        _NC_CACHE[MM_DT_NAME] = _build(mybir.dt(MM_DT_NAME))
    return _NC_CACHE[MM_DT_NAME]


def make_in_maps(x, W_qkv, W_o, b_o):
    x = np.asarray(x, np.float32)
    W_qkv = np.asarray(W_qkv, np.float32)
    W_o = np.asarray(W_o, np.float32)
    b_o = np.asarray(b_o, np.float32)
    in_maps = []
    for c in range(8):
        bg, hg = divmod(c, 4)
        cs = hg * H_LOC * DH
        in_maps.append(
            {
                "x": np.ascontiguousarray(x[B_LOC * bg : B_LOC * (bg + 1)]),
                "wq": np.ascontiguousarray(W_qkv[:, cs : cs + H_LOC * DH]),
                "wk": np.ascontiguousarray(W_qkv[:, D + cs : D + cs + H_LOC * DH]),
                "wv": np.ascontiguousarray(
                    W_qkv[:, 2 * D + cs : 2 * D + cs + H_LOC * DH]
                ),
                "wo": np.ascontiguousarray(W_o),
                "bo": np.ascontiguousarray(b_o.reshape(1, D)),
            }
        )
    return in_maps


def assemble(results) -> np.ndarray:
    out = np.empty((B, N, D), np.float32)
    for c, r in enumerate(results):
        bg, hg = divmod(c, 4)
        out[B_LOC * bg : B_LOC * (bg + 1), hg * 512 : (hg + 1) * 512, :] = r["y"]
    return out


def kernel(x, W_qkv, W_o, b_o) -> np.ndarray:
    nc = _get_nc()
    in_maps = make_in_maps(x, W_qkv, W_o, b_o)
    res = run_bass_kernel_spmd(nc, in_maps, core_ids=list(range(8)))
    return assemble(res.results)
